# revision 3
# baseline (speedup 1.0000x reference)
"""Trainium2 Bass kernel for nn_CustomCIFAR10Model.

Math (reference):
    xf = x.reshape(B, D)
    part2[b,d] = cos(xf[b,d]) * Sa[d] + sin(xf[b,d]) * Sb[d]
        where Sa[d] = sum_i a[i,d,0], Sb[d] = sum_i b[i,d,0]
    part1 = sum(w[1:]*n[1:] + w[:-1]*n[:-1])            (scalar)
    out = (part1 + part2) @ fc_w.T + fc_b               [B, NCLS]

The heavy part is reading a and b (2 x 37.75 MB) once to column-sum them:
memory-bound. Sharding: columns (d) split across 8 cores, 384 each. Every
core independently column-sums its a/b slice, builds z = cos*Sa + sin*Sb
for its d-slice, and contracts against its fc_w columns, yielding a partial
[NCLS, B] output. Host sums the 8 partials and adds part1/bias terms
(part1 contributes part1 * rowsum(fc_w) to every batch row).
"""

import numpy as np

B = 512
D = 3072
NCLS = 100
P = 128
NCORES = 8
DW = D // NCORES          # 384 columns per core
NSUB = DW // P            # 3 d-subtiles of 128
NCH = D // P              # 24 row-chunks of a/b slice

_F32 = None  # set lazily (mybir import is heavy)
_STATE = {}


def _build():
    """Build + bacc-compile the SPMD Bass program (once per process)."""
    import concourse.bacc as bacc
    import concourse.mybir as mybir
    import concourse.tile as tile

    f32 = mybir.dt.float32
    nc = bacc.Bacc(
        "TRN2", target_bir_lowering=False, debug=False, num_devices=NCORES
    )

    a_s = nc.dram_tensor("a_s", [D, DW], f32, kind="ExternalInput")
    b_s = nc.dram_tensor("b_s", [D, DW], f32, kind="ExternalInput")
    xt_s = nc.dram_tensor("xt_s", [DW, B], f32, kind="ExternalInput")
    fwt_s = nc.dram_tensor("fwt_s", [DW, NCLS], f32, kind="ExternalInput")
    out_cb = nc.dram_tensor("out_cb", [NCLS, B], f32, kind="ExternalOutput")

    with tile.TileContext(nc) as tc:
        with (
            tc.tile_pool(name="chunks", bufs=8) as chunk_pool,
            tc.tile_pool(name="accs", bufs=1) as acc_pool,
            tc.tile_pool(name="consts", bufs=1) as const_pool,
            tc.tile_pool(name="xwork", bufs=2) as x_pool,
            tc.tile_pool(name="wpool", bufs=3) as w_pool,
            tc.tile_pool(name="outp", bufs=1) as out_pool,
            tc.tile_pool(name="ps", bufs=2, space="PSUM") as psum_pool,
            tc.tile_pool(name="psout", bufs=1, space="PSUM") as psum_out_pool,
        ):
            ones = const_pool.tile([P, 1], f32, name="ones")
            nc.vector.memset(ones[:], 1.0)
            zero = const_pool.tile([P, 1], f32, name="zerob")
            nc.vector.memset(zero[:], 0.0)
            # cols[:, 0:3] = Sa per subtile, cols[:, 3:6] = Sb per subtile
            cols = const_pool.tile([P, 6], f32, name="cols")

            # Phase 1: acc[p, d] = sum_c T[c*128 + p, d], then PE-reduce
            # over partitions with a ones vector -> per-partition Sa/Sb.
            for ti, src in enumerate((a_s, b_s)):
                acc = acc_pool.tile([P, DW], f32, name=f"acc{ti}", tag=f"acc{ti}")
                for c in range(NCH):
                    ch = chunk_pool.tile(
                        [P, DW], f32, name=f"ch{ti}_{c}", tag="chunk"
                    )
                    nc.sync.dma_start(out=ch[:], in_=src[c * P : (c + 1) * P, :])
                    if c == 0:
                        nc.vector.tensor_copy(acc[:], ch[:])
                    else:
                        nc.vector.tensor_add(acc[:], acc[:], ch[:])
                for sub in range(NSUB):
                    ps = psum_pool.tile([P, 1], f32, name=f"ps{ti}_{sub}", tag="ps")
                    nc.tensor.matmul(
                        ps[:],
                        acc[:, sub * P : (sub + 1) * P],
                        ones[:],
                        start=True,
                        stop=True,
                    )
                    nc.vector.tensor_copy(cols[:, 3 * ti + sub : 3 * ti + sub + 1], ps[:])

            # Phase 2: z = cos(x)*Sa + sin(x)*Sb per d-subtile (d on
            # partitions), then contract over d with fc_w columns on PE.
            # HW Sin only accepts [-pi, pi]; reduce t = x/(2pi) mod 1 via
            # the fp32 magic-round trick, then Sin(2pi * r).
            INV2PI = float(1.0 / (2.0 * np.pi))
            TWO_PI = float(2.0 * np.pi)
            MAGIC = float(1.5 * 2.0**23)
            add_op = mybir.AluOpType.add
            sub_op = mybir.AluOpType.subtract
            mult_op = mybir.AluOpType.mult
            out_ps = psum_out_pool.tile([NCLS, B], f32, name="out_ps")
            for sub in range(NSUB):
                xt = x_pool.tile([P, B], f32, name=f"xt{sub}", tag="xt")
                nc.sync.dma_start(out=xt[:], in_=xt_s[sub * P : (sub + 1) * P, :])
                # sin path: t = x/(2pi); r = t - round(t); sin = Sin(2pi*r)
                ts_t = x_pool.tile([P, B], f32, name=f"ts{sub}", tag="ts")
                nc.vector.tensor_scalar_mul(ts_t[:], xt[:], INV2PI)
                ks_t = x_pool.tile([P, B], f32, name=f"ks{sub}", tag="ks")
                nc.vector.tensor_scalar(ks_t[:], ts_t[:], MAGIC, MAGIC, add_op, sub_op)
                nc.vector.tensor_sub(ts_t[:], ts_t[:], ks_t[:])
                sinv = x_pool.tile([P, B], f32, name=f"sin{sub}", tag="sin")
                nc.scalar.activation(
                    sinv[:], ts_t[:], mybir.ActivationFunctionType.Sin,
                    bias=zero[:], scale=TWO_PI,
                )
                # cos path: t = x/(2pi) + 1/4; r = t - round(t); cos = Sin(2pi*r)
                tc_t = x_pool.tile([P, B], f32, name=f"tc{sub}", tag="tc")
                nc.vector.tensor_scalar(tc_t[:], xt[:], INV2PI, 0.25, mult_op, add_op)
                kc_t = x_pool.tile([P, B], f32, name=f"kc{sub}", tag="kc")
                nc.vector.tensor_scalar(kc_t[:], tc_t[:], MAGIC, MAGIC, add_op, sub_op)
                nc.vector.tensor_sub(tc_t[:], tc_t[:], kc_t[:])
                cosv = x_pool.tile([P, B], f32, name=f"cos{sub}", tag="cos")
                nc.scalar.activation(
                    cosv[:], tc_t[:], mybir.ActivationFunctionType.Sin,
                    bias=zero[:], scale=TWO_PI,
                )
                nc.vector.tensor_scalar_mul(cosv[:], cosv[:], cols[:, sub : sub + 1])
                nc.vector.tensor_scalar_mul(sinv[:], sinv[:], cols[:, 3 + sub : 4 + sub])
                nc.vector.tensor_add(cosv[:], cosv[:], sinv[:])
                fwt = w_pool.tile([P, NCLS], f32, name=f"fwt{sub}", tag="fwt")
                nc.sync.dma_start(out=fwt[:], in_=fwt_s[sub * P : (sub + 1) * P, :])
                nc.tensor.matmul(
                    out_ps[:],
                    fwt[:],
                    cosv[:],
                    start=(sub == 0),
                    stop=(sub == NSUB - 1),
                )

            out_sb = out_pool.tile([NCLS, B], f32, name="out_sb")
            nc.vector.tensor_copy(out_sb[:], out_ps[:])
            nc.sync.dma_start(out=out_cb[:], in_=out_sb[:])

    nc.compile()
    return nc


def _get_nc():
    if "nc" not in _STATE:
        _STATE["nc"] = _build()
    return _STATE["nc"]


def _prep_in_maps(x, a, b, fc_w):
    xf = np.ascontiguousarray(np.asarray(x, dtype=np.float32).reshape(B, D))
    xt = np.ascontiguousarray(xf.T)  # [D, B]
    a2 = np.asarray(a, dtype=np.float32).reshape(D, D)
    b2 = np.asarray(b, dtype=np.float32).reshape(D, D)
    fw = np.asarray(fc_w, dtype=np.float32)
    in_maps = []
    for m in range(NCORES):
        sl = slice(m * DW, (m + 1) * DW)
        in_maps.append(
            {
                "a_s": np.ascontiguousarray(a2[:, sl]),
                "b_s": np.ascontiguousarray(b2[:, sl]),
                "xt_s": np.ascontiguousarray(xt[sl, :]),
                "fwt_s": np.ascontiguousarray(fw[:, sl].T),
            }
        )
    return in_maps


def _run(inputs, trace=False, trace_kwargs=None):
    """Run the device kernel; returns (final_output, BassKernelResults)."""
    from concourse.bass_utils import run_bass_kernel_spmd

    x = inputs["x"]
    a = inputs["a"]
    b = inputs["b"]
    w = np.asarray(inputs["w"], dtype=np.float64)
    n_param = np.asarray(inputs["n_param"], dtype=np.float64)
    fc_w = np.asarray(inputs["fc_w"], dtype=np.float32)
    fc_b = np.asarray(inputs["fc_b"], dtype=np.float32)

    nc = _get_nc()
    in_maps = _prep_in_maps(x, a, b, fc_w)
    res = run_bass_kernel_spmd(
        nc,
        in_maps,
        list(range(NCORES)),
        trace=trace,
        **(trace_kwargs or {}),
    )

    acc = np.zeros((NCLS, B), dtype=np.float32)
    for r in res.results:
        acc += r["out_cb"]
    part1 = float(np.sum(w[1:] * n_param[1:] + w[:-1] * n_param[:-1]))
    final = acc.T + np.float32(part1) * fc_w.sum(axis=1)[None, :] + fc_b[None, :]
    return np.ascontiguousarray(final.astype(np.float32)), res


def kernel(**inputs) -> np.ndarray:
    out, _ = _run(inputs, trace=False)
    return out
